# revision 1
# baseline (speedup 1.0000x reference)
"""Trainium2 Bass kernel for CoA co-attention:

    out[b, i, j] = sum_h a[h] * tanh((cell @ w_k)[b,i,h] + (drug @ w_q)[b,j,h] + bias[h])

Shapes: cell/drug [8, 1024, 64], w_q/w_k [64, 32], bias/a [32] -> out [8, 1024, 1024].

Strategy: fully data-parallel over the batch dim (8 cores, one batch slice
each). Per core:
  - sign-fold: a*tanh(e) = |a|*tanh(sign(a)*e); sign(a) folded into w_k/w_q/bias
    columns on the host so the device only needs |a|.
  - drug projection computed directly into a 4x-partition-replicated layout
    D4[32g+h, j] = (drug @ w_q')[j, h] via a horizontally tiled weight.
  - cell projection + bias computed in a "grouped" layout
    CB[32g+h, t] = (cell @ w_k')[4t+g, h] + bias'[h] (4 cell rows per column).
  - main loop over 256 groups t (4 cell rows each): DVE per-partition-scalar
    add e = D4 + CB[:, t]; ACT tanh (the roofline engine) in big batched
    instructions; PE contracts over h with a block-diagonal |a| matrix (bf16),
    accumulating 8 groups (32 output rows) per PSUM bank before evacuation.
"""

import sys

for p in ("/opt/trn_rl_repo",):
    if p not in sys.path:
        sys.path.insert(0, p)

import numpy as np
import ml_dtypes

from concourse import bass, bacc, tile, mybir
from concourse.bass_utils import run_bass_kernel_spmd

F32 = mybir.dt.float32
BF16 = mybir.dt.bfloat16

B, N, D, H = 8, 1024, 64, 32
G4 = 4           # cell rows per group (128 partitions / 32 h)
NGRP = N // G4   # 256 groups
BAND = 8         # groups accumulated per psum quarter (32 output rows)
NBAND = NGRP // BAND  # 32
ACTG = 16        # groups per ACT instruction (2 bands)

_CACHE = {}


def build_nc():
    nc = bacc.Bacc("TRN2", target_bir_lowering=False, debug=False)

    cellg_d = nc.dram_tensor("cellg", [D + 1, N], F32, kind="ExternalInput")
    drugT_d = nc.dram_tensor("drugT", [D, N], BF16, kind="ExternalInput")
    wks_d = nc.dram_tensor("wks", [D + 1, H], F32, kind="ExternalInput")
    wqs4_d = nc.dram_tensor("wqs4", [D, 4 * H], BF16, kind="ExternalInput")
    a32_d = nc.dram_tensor("a32", [128, 256], BF16, kind="ExternalInput")
    out_d = nc.dram_tensor("out", [N, N], F32, kind="ExternalOutput")

    with tile.TileContext(nc) as tc:
        with (
            tc.tile_pool(name="const", bufs=1) as cpool,
            tc.tile_pool(name="esup", bufs=2) as epool,
            tc.tile_pool(name="tsup", bufs=2) as tpool,
            tc.tile_pool(name="osb", bufs=2) as opool,
            tc.tile_pool(name="psA", bufs=2, space=bass.MemorySpace.PSUM) as psA,
            tc.tile_pool(name="psB", bufs=4, space=bass.MemorySpace.PSUM) as psB,
        ):
            # ---- load inputs -------------------------------------------------
            cellg_sb = cpool.tile([D + 1, N], F32, tag="cellg")
            drugT_sb = cpool.tile([D, N], BF16, tag="drugT")
            wks_sb = cpool.tile([D + 1, H], F32, tag="wks")
            wqs4_sb = cpool.tile([D, 4 * H], BF16, tag="wqs4")
            a32_sb = cpool.tile([128, 256], BF16, tag="a32")
            nc.sync.dma_start(out=a32_sb[:], in_=a32_d[:])
            nc.sync.dma_start(out=wqs4_sb[:], in_=wqs4_d[:])
            nc.sync.dma_start(out=drugT_sb[:, :512], in_=drugT_d[:, :512])
            nc.sync.dma_start(out=drugT_sb[:, 512:], in_=drugT_d[:, 512:])
            nc.scalar.dma_start(out=wks_sb[:], in_=wks_d[:])
            nc.scalar.dma_start(out=cellg_sb[:, :512], in_=cellg_d[:, :512])
            nc.scalar.dma_start(out=cellg_sb[:, 512:], in_=cellg_d[:, 512:])

            # PE HAM warm-up: ~3.5us of dummy matmuls on a32 (first DMA to
            # land) while the big inputs stream in, so the fp32 projection
            # matmuls below run at 2.4 GHz instead of the cold 1.2 GHz.
            warm = psA.tile([32, 256], F32, tag="pb", name="warm")
            for i in range(16):
                nc.tensor.matmul(
                    warm[:, :], a32_sb[:, :32], a32_sb[:, :],
                    start=True, stop=True,
                )

            # ---- projections -------------------------------------------------
            # D4[32g+h, j] = drug_attn_T[h, j] (replicated over g), stored bf16
            # so the e-add runs in the DVE's 4x perf mode (tanh-output bf16
            # rounding dominates the error budget either way).
            d4_sb = cpool.tile([128, N], BF16, tag="d4")
            for jh in range(2):
                pd = psA.tile([128, 512], F32, tag="pd")
                nc.tensor.matmul(
                    pd[:, :], wqs4_sb[:, :], drugT_sb[:, 512 * jh:512 * (jh + 1)],
                    start=True, stop=True,
                )
                nc.vector.tensor_copy(d4_sb[:, 512 * jh:512 * (jh + 1)], pd[:, :])

            # CB[32g+h, t] = cell_attn_T[h, 4t+g] + bias'[h]
            # cellg free layout: column (g*256 + t) holds cell row i = 4t+g
            # (host pre-grouped); row 64 of cellg is ones, row 64 of wks is bias'.
            cb_sb = cpool.tile([128, NGRP], F32, tag="cb")
            for g in range(4):
                pb = psA.tile([32, NGRP], F32, tag="pb")
                nc.tensor.matmul(
                    pb[:, :], wks_sb[:, :], cellg_sb[:, NGRP * g:NGRP * (g + 1)],
                    start=True, stop=True,
                )
                nc.vector.tensor_copy(cb_sb[32 * g:32 * (g + 1), :], pb[:, :])

            # ---- main loop ---------------------------------------------------
            # super = 16 groups = 2 bands; band = 8 groups = 32 output rows;
            # macro-band = 4 bands = 128 rows. Matmuls for band q of a
            # macro-band col-tile into psum partitions [32q:32q+32], so each
            # [128, 512] psum bank holds 128 output rows -> one full-lane DVE
            # evacuation per jh per macro-band.
            NSUP = NGRP // ACTG
            for sup in range(NSUP):
                e_sup = epool.tile([128, ACTG * N], BF16, tag="esup")
                t_sup = tpool.tile([128, ACTG * N], BF16, tag="tsup")
                for u in range(ACTG):
                    t = ACTG * sup + u
                    nc.vector.tensor_scalar_add(
                        e_sup[:, N * u:N * (u + 1)], d4_sb[:, :],
                        cb_sb[:, t:t + 1],
                    )
                # First super is on the critical path at startup: split its ACT
                # so tanh begins after only 2 DVE adds. The last super is split
                # so the final matmuls overlap the ACT tail.
                if sup == 0:
                    chunks = ((0, 1), (1, 3), (3, 8), (8, 16))
                elif sup == NSUP - 1:
                    chunks = ((0, 8), (8, 14), (14, 16))
                else:
                    chunks = ((0, 16),)
                for lo, hi in chunks:
                    nc.scalar.activation(
                        t_sup[:, N * lo:N * hi], e_sup[:, N * lo:N * hi],
                        mybir.ActivationFunctionType.Tanh,
                    )

                for p in range(2):
                    band = 2 * sup + p
                    q = band % 4
                    if q == 0:
                        pos = [
                            psB.tile([128, 512], F32, tag="po",
                                     name=f"po{band}_{j}")
                            for j in range(2)
                        ]
                    for jh in range(2):
                        po = pos[jh]
                        for u in range(BAND):
                            g = BAND * p + u
                            rhs = t_sup[:, N * g + 512 * jh:
                                        N * g + 512 * (jh + 1)]
                            nc.tensor.matmul(
                                po[32 * q:32 * (q + 1), :],
                                a32_sb[:, 32 * u:32 * (u + 1)], rhs,
                                start=(u == 0), stop=(u == BAND - 1),
                                tile_position=(0, 32 * q),
                            )
                    if q == 3:
                        mb = band // 4
                        out_sb = opool.tile([128, N], F32, tag="osb")
                        for jh in range(2):
                            nc.vector.tensor_copy(
                                out_sb[:, 512 * jh:512 * (jh + 1)], pos[jh][:, :]
                            )
                            nc.sync.dma_start(
                                out=out_d[128 * mb:128 * (mb + 1),
                                          512 * jh:512 * (jh + 1)],
                                in_=out_sb[:, 512 * jh:512 * (jh + 1)],
                            )
    nc.compile()
    return nc


def _host_prep(cell, drug, w_q, w_k, bias, a):
    """Host-side sharding prep: sign-folding + layout shuffles (no projections)."""
    a = np.asarray(a, np.float32)
    s = np.where(a < 0, -1.0, 1.0).astype(np.float32)
    aabs = np.abs(a).astype(np.float32)

    wks = np.concatenate(
        [np.asarray(w_k, np.float32) * s[None, :], (np.asarray(bias, np.float32) * s)[None, :]],
        axis=0,
    )  # [65, 32]
    wqs = np.asarray(w_q, np.float32) * s[None, :]  # [64, 32]
    # drug side runs as a bf16 matmul: D4 is stored bf16 anyway, so the extra
    # input rounding is ~0.4e-3 on the final result.
    wqs4 = np.ascontiguousarray(np.tile(wqs, (1, 4))).astype(ml_dtypes.bfloat16)

    # a32[:, 32u:32u+32] is variant u: a32[32g+h, 32u + 4u+g] = |a[h]|
    a32 = np.zeros((128, 256), np.float32)
    for u in range(8):
        for g in range(4):
            a32[32 * g:32 * (g + 1), 32 * u + 4 * u + g] = aabs
    a32 = a32.astype(ml_dtypes.bfloat16)

    in_maps = []
    for b in range(B):
        cT = np.asarray(cell[b], np.float32).T  # [64, 1024]
        # grouped: column (g*256 + t) = cell row 4t+g
        cg = cT.reshape(D, NGRP, G4).transpose(0, 2, 1).reshape(D, N)
        cellg = np.concatenate([cg, np.ones((1, N), np.float32)], axis=0)
        cellg = np.ascontiguousarray(cellg)
        drugT = np.ascontiguousarray(np.asarray(drug[b], np.float32).T).astype(ml_dtypes.bfloat16)
        in_maps.append(
            {"cellg": cellg, "drugT": drugT, "wks": wks, "wqs4": wqs4, "a32": a32}
        )
    return in_maps


def kernel(cell, drug, w_q, w_k, bias, a, _trace=False):
    if "nc" not in _CACHE:
        _CACHE["nc"] = build_nc()
    nc = _CACHE["nc"]
    in_maps = _host_prep(cell, drug, w_q, w_k, bias, a)
    try:
        res = run_bass_kernel_spmd(nc, in_maps, list(range(B)), trace=_trace)
    except Exception:
        # one retry for transient device errors (e.g. NRT exec-unit hiccups)
        res = run_bass_kernel_spmd(nc, in_maps, list(range(B)), trace=_trace)
    out = np.stack([np.asarray(res.results[i]["out"]) for i in range(B)], axis=0)
    if _trace:
        _CACHE["last_results"] = res
    return out.astype(np.float32)

